# revision 4
# baseline (speedup 1.0000x reference)
"""APPNP GNN distributed Bass kernel for TRN2 (8 NeuronCores).

v3 design (from v2):
  - Row (destination-node) 1D sharding: core c owns rows [c*R, (c+1)*R).
  - Gather table [N, 128] bf16 (64 real features + 64 zero pad -> 256B rows)
    replicated per-core in DRAM, refreshed each APPNP step by AllGather.
  - Messages land in bf16 directly from dma_gather (no convert pass);
    TensorE consumes them as the moving operand against the bf16 S matrix.
  - One dma_gather call per (chunk, side): ~2.4-2.6K descriptors per call
    amortizes the ~1us SWDGE fixed overhead (vs 1K-desc calls in v2).
  - MLP tail writes alpha*h0 straight into SBUF h0s (no DRAM round trip)
    and a bf16 padded row tile that is DMA'd to hnew for the AllGather.
  - 3-layer MLP on TensorEngine, fp32, activations feature-major.
"""
from contextlib import ExitStack
from dataclasses import dataclass
import numpy as np
import ml_dtypes

from concourse import bass, bacc, mybir, library_config

FP = mybir.dt.float32
BF = mybir.dt.bfloat16
I16 = mybir.dt.int16
AF = mybir.ActivationFunctionType


@dataclass
class Cfg:
    N: int = 65536
    CORES: int = 8
    IN: int = 512           # padded input dim (real 500)
    HID: int = 256
    D: int = 64
    DP: int = 128           # padded feature dim for 256B gather rows
    K: int = 10
    ALPHA: float = 0.1
    BPC: int = 2            # dest blocks per chunk
    WLO: int = 0            # windows per block, LO side (filled by prep)
    WHI: int = 0
    DEBUG: bool = False

    @property
    def R(self):
        return self.N // self.CORES

    @property
    def NB(self):           # dest blocks per core
        return self.R // 128

    @property
    def WPB(self):
        return self.WLO + self.WHI

    @property
    def NWIN(self):         # windows per core
        return self.NB * self.WPB

    @property
    def S_SLOTS(self):      # gather slots per core
        return self.NWIN * 128

    @property
    def NCHUNK(self):
        return self.NB // self.BPC


def wrap16(a):
    m = a.reshape(-1, 16).T
    return np.tile(m, (8, 1)).copy()


def prep_inputs(cfg, x, W1, b1, W2, b2, W3, b3, edge_weight, edge_row, edge_col):
    N, R, D = cfg.N, cfg.R, cfg.D
    HALF = N // 2
    edge_row = np.asarray(edge_row).astype(np.int64)
    edge_col = np.asarray(edge_col).astype(np.int64)
    edge_weight = np.asarray(edge_weight).astype(np.float32)
    x = np.asarray(x)

    # global sort once: by (block=row//128, side=col>=HALF)
    blk = edge_row // 128                       # global block id
    side = (edge_col >= HALF).astype(np.int64)
    order = np.lexsort((edge_col, side, blk))
    er, ec, ew, sd = edge_row[order], edge_col[order], edge_weight[order], side[order]
    gblk = blk[order]

    NBG = N // 128                              # total blocks
    cnt = np.zeros((NBG, 2), np.int64)
    np.add.at(cnt, (gblk, sd), 1)
    cfg.WLO = max(int(np.ceil(cnt[:, 0].max() / 128)), 1)
    cfg.WHI = max(int(np.ceil(cnt[:, 1].max() / 128)), 1)

    NB, BPC, WLO, WHI, WPB = cfg.NB, cfg.BPC, cfg.WLO, cfg.WHI, cfg.WPB
    assert NB % BPC == 0
    CH2 = BPC * WPB * 128

    eye = np.eye(128, dtype=np.float32)
    W1p = np.zeros((cfg.IN, cfg.HID), np.float32)
    W1p[:W1.shape[0]] = W1

    # per-edge slot id within its core:
    #   chunk base + LO: brel*WLO*128 + rank | HI: BPC*WLO*128 + brel*WHI*128 + rank
    b_loc = gblk % NB
    chunk = b_loc // BPC
    brel = b_loc % BPC
    grp = gblk * 2 + sd
    grp_starts = np.searchsorted(grp, np.arange(NBG * 2), side="left")
    rank = np.arange(len(er)) - grp_starts[grp]
    slot = (chunk * CH2
            + np.where(sd == 0,
                       brel * WLO * 128 + rank,
                       BPC * WLO * 128 + brel * WHI * 128 + rank))

    core = gblk // NB
    dest_rel = er % 128
    gval = np.where(sd == 0, ec, ec - HALF).astype(np.int16)

    S_SLOTS = cfg.S_SLOTS
    p_arr = (np.arange(S_SLOTS) % 128).astype(np.int64)
    w_arr = (np.arange(S_SLOTS) // 128).astype(np.int64)
    in_maps = []
    for c in range(cfg.CORES):
        m = core == c
        sl = slot[m].astype(np.int64)
        assert sl.max() < S_SLOTS
        gidx = np.zeros(S_SLOTS, np.int16)
        gidx[sl] = gval[m]
        drel = np.zeros(S_SLOTS, np.int64)
        drel[sl] = dest_rel[m]
        wt = np.zeros(S_SLOTS, np.float32)
        wt[sl] = ew[m]

        S = np.zeros((128, cfg.NWIN, 128), ml_dtypes.bfloat16)
        S[p_arr, w_arr, drel] = wt.astype(ml_dtypes.bfloat16)

        xT = np.zeros((cfg.IN, R), np.float32)
        xs = np.asarray(x[c * R:(c + 1) * R])
        xT[:xs.shape[1], :] = xs.T.astype(np.float32)

        in_maps.append({
            "xT": np.ascontiguousarray(xT),
            "W1": W1p,
            "b1": np.asarray(b1).astype(np.float32).reshape(-1, 128).T.copy(),
            "W2": np.asarray(W2).astype(np.float32),
            "b2": np.asarray(b2).astype(np.float32).reshape(-1, 128).T.copy(),
            "W3": np.asarray(W3).astype(np.float32),
            "b3": np.asarray(b3).reshape(-1, 1).astype(np.float32),
            "eye": eye,
            "gidx": wrap16(gidx),
            "smat": S,
        })
    return cfg, in_maps


def build(cfg: Cfg):
    N, R, D, K = cfg.N, cfg.R, cfg.D, cfg.K
    HALF = N // 2
    IN, HID, DP = cfg.IN, cfg.HID, cfg.DP
    KIN, KH, MH = IN // 128, HID // 128, HID // 128
    NT = R // 128
    NB, BPC, WLO, WHI, WPB = cfg.NB, cfg.BPC, cfg.WLO, cfg.WHI, cfg.WPB
    NWC = BPC * WPB
    CH2 = NWC * 128
    NLOW = BPC * WLO
    NHIW = BPC * WHI
    NCH = cfg.NCHUNK
    FPB = NB * D

    nc = bacc.Bacc(target_bir_lowering=False, num_devices=cfg.CORES,
                   num_swdge_queues=4)

    xT = nc.declare_dram_parameter("xT", [IN, R], FP, isOutput=False)
    W1 = nc.declare_dram_parameter("W1", [IN, HID], FP, isOutput=False)
    b1 = nc.declare_dram_parameter("b1", [128, HID // 128], FP, isOutput=False)
    W2 = nc.declare_dram_parameter("W2", [HID, HID], FP, isOutput=False)
    b2 = nc.declare_dram_parameter("b2", [128, HID // 128], FP, isOutput=False)
    W3 = nc.declare_dram_parameter("W3", [HID, D], FP, isOutput=False)
    b3 = nc.declare_dram_parameter("b3", [D, 1], FP, isOutput=False)
    eye = nc.declare_dram_parameter("eye", [128, 128], FP, isOutput=False)
    gidx = nc.declare_dram_parameter("gidx", [128, cfg.S_SLOTS // 16], I16, isOutput=False)
    smat = nc.declare_dram_parameter("smat", [128, cfg.NWIN, 128], BF, isOutput=False)
    out = nc.declare_dram_parameter("out", [R, D], FP, isOutput=True)

    table = nc.dram_tensor("table", [N, DP], BF, addr_space="Shared")
    hnew = nc.dram_tensor("hnew", [R, DP], BF)

    # ---- semaphore plan (every DMA sem has <=1 DMA in flight) ----
    # smain: sync-engine uploads + x tiles (chained)
    # sd0/sd1: S-tile DMAs per parity (chained via matmul-consumption waits)
    # gmain: gpsimd misc DMAs (hnew row writes, step writes) chained
    # gL0/gL1/gH0/gH1: gather DMAs per parity+side (ordered via matmul waits)
    # v/a/p: compute sems (in-order per engine); c: collectives
    GATHER_SEMS = [f"g{side}{par}" for side in "LH" for par in range(2)]
    SEMNAMES = ["smain", "sd0", "sd1", "gmain"] + GATHER_SEMS + ["v", "a", "p", "c"]
    DMA_SEMS = {"smain", "sd0", "sd1", "gmain", *GATHER_SEMS}
    ENG_OF = {sn: 'g' for sn in GATHER_SEMS}
    ENG_OF.update({"smain": 's', "sd0": 's', "sd1": 's', "gmain": 'g',
                   "v": 'v', "a": 'a', "p": 'p', "c": 'g'})
    sched = []      # (engine, fn, waits{semname: val}, semname)
    cnt = {sn: 0 for sn in SEMNAMES}

    def add(semname, fn, waits=None):
        sched.append((ENG_OF[semname], fn, dict(waits or {}), semname))
        cnt[semname] += 16 if semname in DMA_SEMS else 1
        return cnt[semname]

    es = ExitStack()
    with es:
        SEMH = {sn: es.enter_context(nc.semaphore("sem_" + sn)) for sn in SEMNAMES}

        gidx_sb = es.enter_context(nc.sbuf_tensor("gidx_sb", [128, cfg.S_SLOTS // 16], I16))
        msgb = [es.enter_context(nc.sbuf_tensor(f"msgb{i}", [128, NWC, DP], BF)) for i in range(2)]
        ssb = [es.enter_context(nc.sbuf_tensor(f"ssb{i}", [128, NWC * 128], BF)) for i in range(2)]
        h0s = es.enter_context(nc.sbuf_tensor("h0s", [128, FPB], FP))
        hnew_sb = es.enter_context(nc.sbuf_tensor("hnew_sb", [128, NB, DP], BF))
        hnu32 = es.enter_context(nc.sbuf_tensor("hnu32", [128, FPB], FP))
        h0bf = es.enter_context(nc.sbuf_tensor("h0bf", [128, DP], BF))
        w1_sb = es.enter_context(nc.sbuf_tensor("w1_sb", [128, KIN, HID], FP))
        w2_sb = es.enter_context(nc.sbuf_tensor("w2_sb", [128, KH, HID], FP))
        w3_sb = es.enter_context(nc.sbuf_tensor("w3_sb", [128, KH, D], FP))
        b1_sb = es.enter_context(nc.sbuf_tensor("b1_sb", [128, MH], FP))
        b2_sb = es.enter_context(nc.sbuf_tensor("b2_sb", [128, MH], FP))
        b3_sb = es.enter_context(nc.sbuf_tensor("b3_sb", [D, 1], FP))
        eye_sb = es.enter_context(nc.sbuf_tensor("eye_sb", [128, 128], FP))
        xt_sb = es.enter_context(nc.sbuf_tensor("xt_sb", [128, KIN, 128], FP))
        h1t_sb = es.enter_context(nc.sbuf_tensor("h1t_sb", [128, KH, 128], FP))
        h2t_sb = es.enter_context(nc.sbuf_tensor("h2t_sb", [128, KH, 128], FP))
        h3t_sb = es.enter_context(nc.sbuf_tensor("h3t_sb", [D, 128], FP))
        ps_a = es.enter_context(nc.psum_tensor("ps_a", [128, 128], FP))
        ps_b = es.enter_context(nc.psum_tensor("ps_b", [128, 128], FP))
        ps_t = es.enter_context(nc.psum_tensor("ps_t", [128, 128], FP))
        ps_blk = [es.enter_context(nc.psum_tensor(f"ps_blk{i}", [128, D], FP))
                  for i in range(2 * BPC)]
        block = es.enter_context(nc.Block())

        # ---------------- uploads (chained on smain) ----------------
        prev_s = 0
        for fn in (
            lambda s: s.dma_start(out=w1_sb[:, :, :], in_=bass.AP(W1, 0, [[HID, 128], [128 * HID, KIN], [1, HID]])),
            lambda s: s.dma_start(out=w2_sb[:, :, :], in_=bass.AP(W2, 0, [[HID, 128], [128 * HID, KH], [1, HID]])),
            lambda s: s.dma_start(out=w3_sb[:, :, :], in_=bass.AP(W3, 0, [[D, 128], [128 * D, KH], [1, D]])),
            lambda s: s.dma_start(out=b1_sb[:, :], in_=b1[:, :]),
            lambda s: s.dma_start(out=b2_sb[:, :], in_=b2[:, :]),
            lambda s: s.dma_start(out=b3_sb[:, :], in_=b3[:, :]),
            lambda s: s.dma_start(out=eye_sb[:, :], in_=eye[:, :]),
            lambda s: s.dma_start(out=gidx_sb[:, :], in_=gidx[:, :]),
        ):
            prev_s = add("smain", fn, {"smain": prev_s})
        UP_TOT = prev_s

        # zero-pad inits (vector engine, no deps)
        add("v", lambda v: v.memset(
            bass.AP(hnew_sb, D, [[NB * DP, 128], [DP, NB], [1, DP - D]]), 0.0))
        add("v", lambda v: v.memset(h0bf[:, D:DP], 0.0))
        V_PAD = cnt["v"]

        # ---------------- MLP (single serial chain) ----------------
        prev = ("smain", UP_TOT)

        def chain(semname, fn, extra=None):
            nonlocal prev
            w = {prev[0]: prev[1]}
            if extra:
                for k2, v2 in extra.items():
                    w[k2] = max(w.get(k2, 0), v2)
            val = add(semname, fn, w)
            prev = (semname, val)

        hnw_prev = 0
        for rt in range(NT):
            chain("smain", lambda s, rt=rt: s.dma_start(
                out=xt_sb[:, :, :],
                in_=bass.AP(xT, rt * 128, [[R, 128], [128 * R, KIN], [1, 128]])))
            for ht in range(MH):
                for kc in range(KIN):
                    chain("p", lambda p, ht=ht, kc=kc: p.matmul(
                        ps_a[:, :],
                        bass.AP(w1_sb, kc * HID + ht * 128, [[KIN * HID, 128], [1, 128]]),
                        xt_sb[:, kc, :],
                        start=(kc == 0), stop=(kc == KIN - 1)))
                chain("a", lambda a, ht=ht: a.activation(
                    h1t_sb[:, ht, :], ps_a[:, :], AF.Relu,
                    bias=b1_sb[:, ht:ht + 1], scale=1.0))
            for ht in range(MH):
                for kc in range(KH):
                    chain("p", lambda p, ht=ht, kc=kc: p.matmul(
                        ps_b[:, :],
                        bass.AP(w2_sb, kc * HID + ht * 128, [[KH * HID, 128], [1, 128]]),
                        h1t_sb[:, kc, :],
                        start=(kc == 0), stop=(kc == KH - 1)))
                chain("a", lambda a, ht=ht: a.activation(
                    h2t_sb[:, ht, :], ps_b[:, :], AF.Relu,
                    bias=b2_sb[:, ht:ht + 1], scale=1.0))
            for kc in range(KH):
                chain("p", lambda p, kc=kc: p.matmul(
                    bass.AP(ps_t, 0, [[128, D], [1, 128]]),
                    bass.AP(w3_sb, kc * D, [[KH * D, 128], [1, D]]),
                    h2t_sb[:, kc, :],
                    start=(kc == 0), stop=(kc == KH - 1)))
            chain("v", lambda v: v.tensor_scalar_add(
                h3t_sb[:, :], bass.AP(ps_t, 0, [[128, D], [1, 128]]), b3_sb[:, :]))
            chain("p", lambda p: p.transpose(
                ps_a[:, 0:D], h3t_sb[:, :], eye_sb[0:D, 0:D]))
            # alpha*h0 straight into SBUF (block rt == row tile rt)
            chain("a", lambda a, rt=rt: a.activation(
                h0s[:, rt * D:(rt + 1) * D], ps_a[:, 0:D], AF.Copy,
                scale=cfg.ALPHA))
            # bf16 padded row tile for the AllGather table
            chain("a", lambda a: a.activation(
                h0bf[:, 0:D], ps_a[:, 0:D], AF.Copy, scale=1.0))
            chain("gmain", lambda g, rt=rt: g.dma_start(
                out=bass.AP(hnew, rt * 128 * DP, [[DP, 128], [1, DP]]),
                in_=h0bf[:, :]), extra={"gmain": hnw_prev, "v": V_PAD})
            hnw_prev = cnt["gmain"]

        A_MLP = cnt["a"]
        G_MLP = cnt["gmain"]

        # ---------------- APPNP steps ----------------
        mm_after_chunk = {}
        flush_v_after_block = {}
        mm_after_block = {}
        gat_cum = {}
        sd_cum = [0, 0]
        hwr_val = G_MLP
        gci = 0   # global chunk counter across steps

        def pending_flush_flush(fgb, fb, fpsum, k):
            if k == K - 1:
                fv = add("v", lambda v, fb=fb, fpsum=fpsum: v.scalar_tensor_tensor(
                    hnu32[:, fb * D:(fb + 1) * D], fpsum[:, :],
                    1.0 - cfg.ALPHA, h0s[:, fb * D:(fb + 1) * D],
                    mybir.AluOpType.mult, mybir.AluOpType.add),
                    {"p": mm_after_block[fgb], "a": A_MLP})
            else:
                fv = add("v", lambda v, fb=fb, fpsum=fpsum: v.scalar_tensor_tensor(
                    hnew_sb[:, fb, 0:D], fpsum[:, :],
                    1.0 - cfg.ALPHA, h0s[:, fb * D:(fb + 1) * D],
                    mybir.AluOpType.mult, mybir.AluOpType.add),
                    {"p": mm_after_block[fgb], "a": A_MLP})
            flush_v_after_block[fgb] = fv

        for k in range(K):
            ag_waits = {"gmain": hwr_val}
            # table reuse: all gathers of previous step done
            for (sidej, par2), val in gat_cum.items():
                ag_waits[f"g{sidej}{par2}"] = val
            add("c", lambda g: g.collective_compute(
                "AllGather", mybir.AluOpType.bypass,
                replica_groups=[list(range(cfg.CORES))],
                ins=[hnew.ap().opt()], outs=[table.ap().opt()]), ag_waits)
            C_NOW = cnt["c"]
            pending_flush = []

            for ci in range(NCH):
                par = gci % 2
                w_g = {"c": C_NOW}
                if mm_after_chunk.get(gci - 2) is not None:
                    w_g["p"] = mm_after_chunk[gci - 2]
                w_mm_gather = {}
                for side, nw_side, base_w, tb_off, qn in (
                    ("L", NLOW, 0, 0, 0), ("H", NHIW, NLOW, HALF * DP, 1),
                ):
                    sn = f"g{side}{par}"
                    gv = add(sn, lambda g, par=par, nw_side=nw_side, base_w=base_w,
                             tb_off=tb_off, ci=ci, qn=qn: g.dma_gather(
                        out_ap=msgb[par][:, base_w:base_w + nw_side, :],
                        in_ap=bass.AP(table, tb_off, [[DP, HALF], [1, DP]]),
                        idxs_ap=gidx_sb[:, (ci * CH2 + base_w * 128) // 16:
                                        (ci * CH2 + (base_w + nw_side) * 128) // 16],
                        num_idxs=nw_side * 128, num_idxs_reg=nw_side * 128,
                        elem_size=DP, queue_num=(2 * par + qn) % 4,
                        single_packet=False), w_g)
                    gat_cum[(side, par)] = gv
                    w_mm_gather[sn] = gv
                w_s = {}
                if mm_after_chunk.get(gci - 2) is not None:
                    w_s["p"] = mm_after_chunk[gci - 2]
                sdv = add("sd" + str(par), lambda s, ci=ci, par=par: s.dma_start(
                    out=ssb[par][:, :],
                    in_=smat[:, ci * NWC:(ci + 1) * NWC, :]), w_s)
                sd_cum[par] = sdv

                for brel in range(BPC):
                    b = ci * BPC + brel
                    gb = k * NB + b
                    psum = ps_blk[((gb // BPC) % 2) * BPC + brel]
                    wins = ([brel * WLO + j for j in range(WLO)]
                            + [NLOW + brel * WHI + j for j in range(WHI)])
                    for wi, w in enumerate(wins):
                        waits = {}
                        if wi == 0:
                            waits = dict(w_mm_gather)
                            waits["sd" + str(par)] = sdv
                            prev_gb = gb - 2 * BPC
                            if prev_gb in flush_v_after_block:
                                waits["v"] = flush_v_after_block[prev_gb]
                        add("p", lambda p, par=par, w=w, psum=psum, wi=wi, nw=len(wins): p.matmul(
                            psum[:, :],
                            bass.AP(ssb[par], w * 128, [[NWC * 128, 128], [1, 128]]),
                            bass.AP(msgb[par], w * DP, [[NWC * DP, 128], [1, D]]),
                            start=(wi == 0), stop=(wi == nw - 1)), waits)
                    mm_after_block[gb] = cnt["p"]
                    pending_flush.append((gb, b, psum))
                    if len(pending_flush) > 1:
                        fgb, fb, fpsum = pending_flush.pop(0)
                        pending_flush_flush(fgb, fb, fpsum, k)
                mm_after_chunk[gci] = cnt["p"]
                gci += 1

            while pending_flush:
                fgb, fb, fpsum = pending_flush.pop(0)
                pending_flush_flush(fgb, fb, fpsum, k)

            if k == K - 1:
                hwr_val = add("gmain", lambda g: g.dma_start(
                    out=bass.AP(out, 0, [[D, 128], [128 * D, NB], [1, D]]),
                    in_=hnu32[:, :]), {"v": cnt["v"], "gmain": hwr_val})
            else:
                hwr_val = add("gmain", lambda g: g.dma_start(
                    out=bass.AP(hnew, 0, [[DP, 128], [128 * DP, NB], [1, DP]]),
                    in_=hnew_sb[:, :, :]), {"v": cnt["v"], "gmain": hwr_val})

        # ------------- emit -------------
        def walk(name):
            def run(eng):
                if name == 'g':
                    eng.load_library(library_config.mlp)
                last = {sn: 0 for sn in SEMNAMES}
                for (e, fn, waits, semname) in sched:
                    if e != name:
                        continue
                    for sk, val in waits.items():
                        if val > last[sk]:
                            eng.wait_ge(SEMH[sk], int(val))
                            last[sk] = int(val)
                    inc = 16 if semname in DMA_SEMS else 1
                    fn(eng).then_inc(SEMH[semname], inc)
                if name == 'g':
                    for sn in SEMNAMES:
                        if cnt[sn] > last[sn]:
                            eng.wait_ge(SEMH[sn], int(cnt[sn]))
            return run

        block.gpsimd(walk('g'))
        block.vector(walk('v'))
        block.sync(walk('s'))
        block.tensor(walk('p'))
        block.scalar(walk('a'))

    return nc


def reference_np(cfg, x, W1, b1, W2, b2, W3, b3, edge_weight, edge_row, edge_col):
    h = np.maximum(x @ W1 + b1, 0)
    h = np.maximum(h @ W2 + b2, 0)
    h = h @ W3 + b3
    h0 = h
    for _ in range(cfg.K):
        msg = h[edge_col] * edge_weight[:, None]
        aggv = np.zeros_like(h0)
        np.add.at(aggv, edge_row, msg)
        h = (1.0 - cfg.ALPHA) * aggv + cfg.ALPHA * h0
    return h


# ----------------------------------------------------------------------------
# Harness entry point: full inputs in, full output out.
# ----------------------------------------------------------------------------
def kernel(**inputs):
    cfg = Cfg()  # full-size defaults
    cfg, in_maps = prep_inputs(
        cfg,
        inputs["x"], inputs["W1"], inputs["b1"], inputs["W2"], inputs["b2"],
        inputs["W3"], inputs["b3"], inputs["edge_weight"],
        inputs["edge_row"], inputs["edge_col"],
    )
    nc = build(cfg)
    nc.finalize()
    from concourse.bass_utils import run_bass_kernel_spmd
    res = run_bass_kernel_spmd(nc, in_maps, core_ids=list(range(cfg.CORES)))
    outs = res.results
    return np.concatenate([o["out"] for o in outs], axis=0).astype(np.float32)


# revision 6
# speedup vs baseline: 1.3146x; 1.3146x over previous
"""APPNP GNN distributed Bass kernel for TRN2 (8 NeuronCores).

v4 design:
  - Row (destination-node) 1D sharding: core c owns rows [c*R, (c+1)*R).
  - Gather table [N, 64] bf16 replicated per-core in DRAM, refreshed each
    APPNP step by AllGather (1 MiB per core in, 8 MiB table out).
  - Pair-fetch gather: each 256B descriptor fetches the bf16 row PAIR
    (2i, 2i+1) with idx = col>>1 (fits int16, no LO/HI split -> fewer
    padded windows). A DVE select (parity mask) picks the right half.
  - dma_gather calls kept at <=1024 descriptors (empirical SWDGE desc-gen
    sweet spot ~2.8ns/desc).
  - Messages aggregated on TensorE: per 128-edge window a host-built
    S matrix [128 edges, 128 dests] (edge weight at the dest column) is
    the stationary operand; PSUM accumulates the segment sum.
  - MLP tail writes alpha*h0 straight into SBUF h0s and a bf16 row tile
    DMA'd to hnew for the AllGather.
"""
from contextlib import ExitStack
from dataclasses import dataclass
import math
import numpy as np
import ml_dtypes

from concourse import bass, bacc, mybir, library_config

FP = mybir.dt.float32
BF = mybir.dt.bfloat16
I16 = mybir.dt.int16
AF = mybir.ActivationFunctionType


@dataclass
class Cfg:
    N: int = 65536
    CORES: int = 8
    IN: int = 512           # padded input dim (real 500)
    HID: int = 256
    D: int = 64
    K: int = 10
    ALPHA: float = 0.1
    BPC: int = 2            # dest blocks per chunk
    WLO: int = 0            # windows per block (filled by prep); WHI kept 0
    WHI: int = 0
    DEBUG: bool = False

    @property
    def R(self):
        return self.N // self.CORES

    @property
    def NB(self):           # dest blocks per core
        return self.R // 128

    @property
    def WPB(self):
        return self.WLO + self.WHI

    @property
    def NWIN(self):         # windows per core
        return self.NB * self.WPB

    @property
    def S_SLOTS(self):      # gather slots per core
        return self.NWIN * 128

    @property
    def NCHUNK(self):
        return self.NB // self.BPC


def wrap16(a):
    m = a.reshape(-1, 16).T
    return np.tile(m, (8, 1)).copy()


def prep_inputs(cfg, x, W1, b1, W2, b2, W3, b3, edge_weight, edge_row, edge_col):
    N, R, D = cfg.N, cfg.R, cfg.D
    edge_row = np.asarray(edge_row).astype(np.int64)
    edge_col = np.asarray(edge_col).astype(np.int64)
    edge_weight = np.asarray(edge_weight).astype(np.float32)
    x = np.asarray(x)

    # global sort once: by dest block
    blk = edge_row // 128                       # global block id
    order = np.lexsort((edge_col, blk))
    er, ec, ew = edge_row[order], edge_col[order], edge_weight[order]
    gblk = blk[order]

    NBG = N // 128                              # total blocks
    cnt = np.zeros(NBG, np.int64)
    np.add.at(cnt, gblk, 1)
    cfg.WLO = max(int(np.ceil(cnt.max() / 128)), 1)
    cfg.WHI = 0

    NB, BPC, W = cfg.NB, cfg.BPC, cfg.WLO
    assert NB % BPC == 0
    CH2 = BPC * W * 128

    eye = np.eye(128, dtype=np.float32)
    W1p = np.zeros((cfg.IN, cfg.HID), np.float32)
    W1p[:W1.shape[0]] = W1

    # per-edge slot id within its core: chunk base + brel*W*128 + rank
    b_loc = gblk % NB
    chunk = b_loc // BPC
    brel = b_loc % BPC
    grp_starts = np.searchsorted(gblk, np.arange(NBG), side="left")
    rank = np.arange(len(er)) - grp_starts[gblk]
    slot = chunk * CH2 + brel * W * 128 + rank

    core = gblk // NB
    dest_rel = er % 128
    gval = (ec >> 1).astype(np.int16)
    pval = (ec & 1).astype(np.float32)

    S_SLOTS = cfg.S_SLOTS
    p_arr = (np.arange(S_SLOTS) % 128).astype(np.int64)
    w_arr = (np.arange(S_SLOTS) // 128).astype(np.int64)
    in_maps = []
    for c in range(cfg.CORES):
        m = core == c
        sl = slot[m].astype(np.int64)
        assert sl.max() < S_SLOTS
        gidx = np.zeros(S_SLOTS, np.int16)
        gidx[sl] = gval[m]
        par = np.zeros(S_SLOTS, np.float32)
        par[sl] = pval[m]
        drel = np.zeros(S_SLOTS, np.int64)
        drel[sl] = dest_rel[m]
        wt = np.zeros(S_SLOTS, np.float32)
        wt[sl] = ew[m]

        S = np.zeros((128, cfg.NWIN, 128), ml_dtypes.bfloat16)
        S[p_arr, w_arr, drel] = wt.astype(ml_dtypes.bfloat16)
        parm = np.zeros((128, cfg.NWIN), np.uint8)
        parm[p_arr, w_arr] = par.astype(np.uint8)

        xT = np.zeros((cfg.IN, R), np.float32)
        xs = np.asarray(x[c * R:(c + 1) * R])
        xT[:xs.shape[1], :] = xs.T.astype(np.float32)

        in_maps.append({
            "xT": np.ascontiguousarray(xT),
            "W1": W1p,
            "b1": np.asarray(b1).astype(np.float32).reshape(-1, 128).T.copy(),
            "W2": np.asarray(W2).astype(np.float32),
            "b2": np.asarray(b2).astype(np.float32).reshape(-1, 128).T.copy(),
            "W3": np.asarray(W3).astype(np.float32),
            "b3": np.asarray(b3).reshape(-1, 1).astype(np.float32),
            "eye": eye,
            "gidx": wrap16(gidx),
            "par": parm,
            "smat": S,
        })
    return cfg, in_maps


def build(cfg: Cfg):
    N, R, D, K = cfg.N, cfg.R, cfg.D, cfg.K
    IN, HID = cfg.IN, cfg.HID
    KIN, KH, MH = IN // 128, HID // 128, HID // 128
    NT = R // 128
    NB, BPC, W = cfg.NB, cfg.BPC, cfg.WLO
    NWC = BPC * W
    CH2 = NWC * 128
    NCH = cfg.NCHUNK
    FPB = NB * D
    MAXW = 8                 # max windows (1024 descs) per dma_gather call
    NCALL = math.ceil(NWC / MAXW)

    nc = bacc.Bacc(target_bir_lowering=False, num_devices=cfg.CORES,
                   num_swdge_queues=4)

    xT = nc.declare_dram_parameter("xT", [IN, R], FP, isOutput=False)
    W1 = nc.declare_dram_parameter("W1", [IN, HID], FP, isOutput=False)
    b1 = nc.declare_dram_parameter("b1", [128, HID // 128], FP, isOutput=False)
    W2 = nc.declare_dram_parameter("W2", [HID, HID], FP, isOutput=False)
    b2 = nc.declare_dram_parameter("b2", [128, HID // 128], FP, isOutput=False)
    W3 = nc.declare_dram_parameter("W3", [HID, D], FP, isOutput=False)
    b3 = nc.declare_dram_parameter("b3", [D, 1], FP, isOutput=False)
    eye = nc.declare_dram_parameter("eye", [128, 128], FP, isOutput=False)
    gidx = nc.declare_dram_parameter("gidx", [128, cfg.S_SLOTS // 16], I16, isOutput=False)
    par = nc.declare_dram_parameter("par", [128, cfg.NWIN], mybir.dt.uint8, isOutput=False)
    smat = nc.declare_dram_parameter("smat", [128, cfg.NWIN, 128], BF, isOutput=False)
    out = nc.declare_dram_parameter("out", [R, D], FP, isOutput=True)

    table = nc.dram_tensor("table", [N, D], BF, addr_space="Shared")
    hnew = nc.dram_tensor("hnew", [R, D], BF)

    # ---- semaphore plan (every DMA sem has <=1 DMA in flight) ----
    GATHER_SEMS = [f"g{p}{j}" for p in range(2) for j in range(NCALL)]
    SEMNAMES = ["smain", "sd0", "sd1", "gmain"] + GATHER_SEMS + ["v", "a", "p", "c"]
    DMA_SEMS = {"smain", "sd0", "sd1", "gmain", *GATHER_SEMS}
    ENG_OF = {sn: 'g' for sn in GATHER_SEMS}
    ENG_OF.update({"smain": 's', "sd0": 's', "sd1": 's', "gmain": 'g',
                   "v": 'v', "a": 'a', "p": 'p', "c": 'g'})
    sched = []      # (engine, fn, waits{semname: val}, semname)
    cnt = {sn: 0 for sn in SEMNAMES}

    def add(semname, fn, waits=None):
        sched.append((ENG_OF[semname], fn, dict(waits or {}), semname))
        cnt[semname] += 16 if semname in DMA_SEMS else 1
        return cnt[semname]

    es = ExitStack()
    with es:
        SEMH = {sn: es.enter_context(nc.semaphore("sem_" + sn)) for sn in SEMNAMES}

        gidx_sb = es.enter_context(nc.sbuf_tensor("gidx_sb", [128, cfg.S_SLOTS // 16], I16))
        par_sb = es.enter_context(nc.sbuf_tensor("par_sb", [128, cfg.NWIN], mybir.dt.uint8))
        msgb = [es.enter_context(nc.sbuf_tensor(f"msgb{i}", [128, NWC, 128], BF)) for i in range(2)]
        msel = [es.enter_context(nc.sbuf_tensor(f"msel{i}", [128, NWC, D], BF)) for i in range(2)]
        ssb = [es.enter_context(nc.sbuf_tensor(f"ssb{i}", [128, NWC * 128], BF)) for i in range(2)]
        h0s = es.enter_context(nc.sbuf_tensor("h0s", [128, FPB], FP))
        hnew_sb = es.enter_context(nc.sbuf_tensor("hnew_sb", [128, FPB], BF))
        hnu32 = es.enter_context(nc.sbuf_tensor("hnu32", [128, FPB], FP))
        h0bf = es.enter_context(nc.sbuf_tensor("h0bf", [128, D], BF))
        w1_sb = es.enter_context(nc.sbuf_tensor("w1_sb", [128, KIN, HID], FP))
        w2_sb = es.enter_context(nc.sbuf_tensor("w2_sb", [128, KH, HID], FP))
        w3_sb = es.enter_context(nc.sbuf_tensor("w3_sb", [128, KH, D], FP))
        b1_sb = es.enter_context(nc.sbuf_tensor("b1_sb", [128, MH], FP))
        b2_sb = es.enter_context(nc.sbuf_tensor("b2_sb", [128, MH], FP))
        b3_sb = es.enter_context(nc.sbuf_tensor("b3_sb", [D, 1], FP))
        eye_sb = es.enter_context(nc.sbuf_tensor("eye_sb", [128, 128], FP))
        xt_sb = es.enter_context(nc.sbuf_tensor("xt_sb", [128, KIN, 128], FP))
        h1t_sb = es.enter_context(nc.sbuf_tensor("h1t_sb", [128, KH, 128], FP))
        h2t_sb = es.enter_context(nc.sbuf_tensor("h2t_sb", [128, KH, 128], FP))
        h3t_sb = es.enter_context(nc.sbuf_tensor("h3t_sb", [D, 128], FP))
        ps_a = es.enter_context(nc.psum_tensor("ps_a", [128, 128], FP))
        ps_b = es.enter_context(nc.psum_tensor("ps_b", [128, 128], FP))
        ps_t = es.enter_context(nc.psum_tensor("ps_t", [128, 128], FP))
        ps_blk = [es.enter_context(nc.psum_tensor(f"ps_blk{i}", [128, D], FP))
                  for i in range(2 * BPC)]
        block = es.enter_context(nc.Block())

        # ---------------- uploads (chained on smain) ----------------
        prev_s = 0
        for fn in (
            lambda s: s.dma_start(out=w1_sb[:, :, :], in_=bass.AP(W1, 0, [[HID, 128], [128 * HID, KIN], [1, HID]])),
            lambda s: s.dma_start(out=w2_sb[:, :, :], in_=bass.AP(W2, 0, [[HID, 128], [128 * HID, KH], [1, HID]])),
            lambda s: s.dma_start(out=w3_sb[:, :, :], in_=bass.AP(W3, 0, [[D, 128], [128 * D, KH], [1, D]])),
            lambda s: s.dma_start(out=b1_sb[:, :], in_=b1[:, :]),
            lambda s: s.dma_start(out=b2_sb[:, :], in_=b2[:, :]),
            lambda s: s.dma_start(out=b3_sb[:, :], in_=b3[:, :]),
            lambda s: s.dma_start(out=eye_sb[:, :], in_=eye[:, :]),
            lambda s: s.dma_start(out=gidx_sb[:, :], in_=gidx[:, :]),
            lambda s: s.dma_start(out=par_sb[:, :], in_=par[:, :]),
        ):
            prev_s = add("smain", fn, {"smain": prev_s})
        UP_TOT = prev_s

        # ---------------- MLP (single serial chain) ----------------
        prev = ("smain", UP_TOT)

        def chain(semname, fn, extra=None):
            nonlocal prev
            w = {prev[0]: prev[1]}
            if extra:
                for k2, v2 in extra.items():
                    w[k2] = max(w.get(k2, 0), v2)
            val = add(semname, fn, w)
            prev = (semname, val)

        hnw_prev = 0
        for rt in range(NT):
            chain("smain", lambda s, rt=rt: s.dma_start(
                out=xt_sb[:, :, :],
                in_=bass.AP(xT, rt * 128, [[R, 128], [128 * R, KIN], [1, 128]])))
            for ht in range(MH):
                for kc in range(KIN):
                    chain("p", lambda p, ht=ht, kc=kc: p.matmul(
                        ps_a[:, :],
                        bass.AP(w1_sb, kc * HID + ht * 128, [[KIN * HID, 128], [1, 128]]),
                        xt_sb[:, kc, :],
                        start=(kc == 0), stop=(kc == KIN - 1)))
                chain("a", lambda a, ht=ht: a.activation(
                    h1t_sb[:, ht, :], ps_a[:, :], AF.Relu,
                    bias=b1_sb[:, ht:ht + 1], scale=1.0))
            for ht in range(MH):
                for kc in range(KH):
                    chain("p", lambda p, ht=ht, kc=kc: p.matmul(
                        ps_b[:, :],
                        bass.AP(w2_sb, kc * HID + ht * 128, [[KH * HID, 128], [1, 128]]),
                        h1t_sb[:, kc, :],
                        start=(kc == 0), stop=(kc == KH - 1)))
                chain("a", lambda a, ht=ht: a.activation(
                    h2t_sb[:, ht, :], ps_b[:, :], AF.Relu,
                    bias=b2_sb[:, ht:ht + 1], scale=1.0))
            for kc in range(KH):
                chain("p", lambda p, kc=kc: p.matmul(
                    bass.AP(ps_t, 0, [[128, D], [1, 128]]),
                    bass.AP(w3_sb, kc * D, [[KH * D, 128], [1, D]]),
                    h2t_sb[:, kc, :],
                    start=(kc == 0), stop=(kc == KH - 1)))
            chain("v", lambda v: v.tensor_scalar_add(
                h3t_sb[:, :], bass.AP(ps_t, 0, [[128, D], [1, 128]]), b3_sb[:, :]))
            chain("p", lambda p: p.transpose(
                ps_a[:, 0:D], h3t_sb[:, :], eye_sb[0:D, 0:D]))
            # alpha*h0 straight into SBUF (block rt == row tile rt)
            chain("a", lambda a, rt=rt: a.activation(
                h0s[:, rt * D:(rt + 1) * D], ps_a[:, 0:D], AF.Copy,
                scale=cfg.ALPHA))
            # bf16 row tile for the AllGather table
            chain("a", lambda a: a.activation(
                h0bf[:, :], ps_a[:, 0:D], AF.Copy, scale=1.0))
            chain("gmain", lambda g, rt=rt: g.dma_start(
                out=bass.AP(hnew, rt * 128 * D, [[D, 128], [1, D]]),
                in_=h0bf[:, :]), extra={"gmain": hnw_prev})
            hnw_prev = cnt["gmain"]

        A_MLP = cnt["a"]
        G_MLP = cnt["gmain"]

        # ---------------- APPNP steps ----------------
        mm_after_chunk = {}
        flush_v_after_block = {}
        mm_after_block = {}
        sel_after_chunk = {}
        gat_cum = {}
        hwr_val = G_MLP
        gci = 0   # global chunk counter across steps

        def emit_flush(fgb, fb, fpsum, k):
            dst = hnu32 if k == K - 1 else hnew_sb
            fv = add("v", lambda v, fb=fb, fpsum=fpsum, dst=dst: v.scalar_tensor_tensor(
                dst[:, fb * D:(fb + 1) * D], fpsum[:, :],
                1.0 - cfg.ALPHA, h0s[:, fb * D:(fb + 1) * D],
                mybir.AluOpType.mult, mybir.AluOpType.add),
                {"p": mm_after_block[fgb], "a": A_MLP})
            flush_v_after_block[fgb] = fv

        for k in range(K):
            ag_waits = {"gmain": hwr_val}
            for (p2, j2), val in gat_cum.items():
                ag_waits[f"g{p2}{j2}"] = val
            add("c", lambda g: g.collective_compute(
                "AllGather", mybir.AluOpType.bypass,
                replica_groups=[list(range(cfg.CORES))],
                ins=[hnew.ap().opt()], outs=[table.ap().opt()]), ag_waits)
            C_NOW = cnt["c"]
            pending_flush = []

            for ci in range(NCH):
                pr = gci % 2
                w_g = {"c": C_NOW}
                if mm_after_chunk.get(gci - 2) is not None:
                    w_g["p"] = mm_after_chunk[gci - 2]
                w_sel_gather = {}
                for j in range(NCALL):
                    w0 = j * MAXW
                    w1 = min(w0 + MAXW, NWC)
                    sn = f"g{pr}{j}"
                    qn = (gci * NCALL + j) % 4
                    gv = add(sn, lambda g, pr=pr, w0=w0, w1=w1, ci=ci, qn=qn: g.dma_gather(
                        out_ap=msgb[pr][:, w0:w1, :],
                        in_ap=bass.AP(table, 0, [[128, N // 2], [1, 128]]),
                        idxs_ap=gidx_sb[:, (ci * CH2 + w0 * 128) // 16:
                                        (ci * CH2 + w1 * 128) // 16],
                        num_idxs=(w1 - w0) * 128, num_idxs_reg=(w1 - w0) * 128,
                        elem_size=128, queue_num=qn,
                        single_packet=False), w_g)
                    gat_cum[(pr, j)] = gv
                    w_sel_gather[sn] = gv
                # parity select: pick odd/even row half per slot
                w_sel = dict(w_sel_gather)
                if mm_after_chunk.get(gci - 2) is not None:
                    w_sel["p"] = mm_after_chunk[gci - 2]
                selv = add("v", lambda v, pr=pr, ci=ci: v.select(
                    msel[pr][:, :, :],
                    bass.AP(par_sb, ci * NWC, [[cfg.NWIN, 128], [1, NWC], [0, D]]),
                    bass.AP(msgb[pr], D, [[NWC * 128, 128], [128, NWC], [1, D]]),
                    bass.AP(msgb[pr], 0, [[NWC * 128, 128], [128, NWC], [1, D]])),
                    w_sel)
                sel_after_chunk[gci] = selv

                w_s = {}
                if mm_after_chunk.get(gci - 2) is not None:
                    w_s["p"] = mm_after_chunk[gci - 2]
                sdv = add("sd" + str(pr), lambda s, ci=ci, pr=pr: s.dma_start(
                    out=ssb[pr][:, :],
                    in_=smat[:, ci * NWC:(ci + 1) * NWC, :]), w_s)

                for brel in range(BPC):
                    b = ci * BPC + brel
                    gb = k * NB + b
                    psum = ps_blk[((gb // BPC) % 2) * BPC + brel]
                    wins = [brel * W + j for j in range(W)]
                    for wi, w in enumerate(wins):
                        waits = {}
                        if wi == 0:
                            waits = {"v": selv, "sd" + str(pr): sdv}
                            prev_gb = gb - 2 * BPC
                            if prev_gb in flush_v_after_block:
                                waits["v"] = max(waits["v"], flush_v_after_block[prev_gb])
                        add("p", lambda p, pr=pr, w=w, psum=psum, wi=wi, nw=len(wins): p.matmul(
                            psum[:, :],
                            bass.AP(ssb[pr], w * 128, [[NWC * 128, 128], [1, 128]]),
                            bass.AP(msel[pr], w * D, [[NWC * D, 128], [1, D]]),
                            start=(wi == 0), stop=(wi == nw - 1)), waits)
                    mm_after_block[gb] = cnt["p"]
                    pending_flush.append((gb, b, psum))
                    if len(pending_flush) > 1:
                        fgb, fb, fpsum = pending_flush.pop(0)
                        emit_flush(fgb, fb, fpsum, k)
                mm_after_chunk[gci] = cnt["p"]
                gci += 1

            while pending_flush:
                fgb, fb, fpsum = pending_flush.pop(0)
                emit_flush(fgb, fb, fpsum, k)

            if k == K - 1:
                hwr_val = add("gmain", lambda g: g.dma_start(
                    out=bass.AP(out, 0, [[D, 128], [128 * D, NB], [1, D]]),
                    in_=hnu32[:, :]), {"v": cnt["v"], "gmain": hwr_val})
            else:
                hwr_val = add("gmain", lambda g: g.dma_start(
                    out=bass.AP(hnew, 0, [[D, 128], [128 * D, NB], [1, D]]),
                    in_=hnew_sb[:, :]), {"v": cnt["v"], "gmain": hwr_val})

        # ------------- emit -------------
        def walk(name):
            def run(eng):
                if name == 'g':
                    eng.load_library(library_config.mlp)
                last = {sn: 0 for sn in SEMNAMES}
                for (e, fn, waits, semname) in sched:
                    if e != name:
                        continue
                    for sk, val in waits.items():
                        if val > last[sk]:
                            eng.wait_ge(SEMH[sk], int(val))
                            last[sk] = int(val)
                    inc = 16 if semname in DMA_SEMS else 1
                    fn(eng).then_inc(SEMH[semname], inc)
                if name == 'g':
                    for sn in SEMNAMES:
                        if cnt[sn] > last[sn]:
                            eng.wait_ge(SEMH[sn], int(cnt[sn]))
            return run

        block.gpsimd(walk('g'))
        block.vector(walk('v'))
        block.sync(walk('s'))
        block.tensor(walk('p'))
        block.scalar(walk('a'))

    return nc


def reference_np(cfg, x, W1, b1, W2, b2, W3, b3, edge_weight, edge_row, edge_col):
    h = np.maximum(x @ W1 + b1, 0)
    h = np.maximum(h @ W2 + b2, 0)
    h = h @ W3 + b3
    h0 = h
    for _ in range(cfg.K):
        msg = h[edge_col] * edge_weight[:, None]
        aggv = np.zeros_like(h0)
        np.add.at(aggv, edge_row, msg)
        h = (1.0 - cfg.ALPHA) * aggv + cfg.ALPHA * h0
    return h


# ----------------------------------------------------------------------------
# Harness entry point: full inputs in, full output out.
# ----------------------------------------------------------------------------
def kernel(**inputs):
    cfg = Cfg()  # full-size defaults
    cfg, in_maps = prep_inputs(
        cfg,
        inputs["x"], inputs["W1"], inputs["b1"], inputs["W2"], inputs["b2"],
        inputs["W3"], inputs["b3"], inputs["edge_weight"],
        inputs["edge_row"], inputs["edge_col"],
    )
    nc = build(cfg)
    nc.finalize()
    from concourse.bass_utils import run_bass_kernel_spmd
    res = run_bass_kernel_spmd(nc, in_maps, core_ids=list(range(cfg.CORES)))
    outs = res.results
    return np.concatenate([o["out"] for o in outs], axis=0).astype(np.float32)


# revision 8
# speedup vs baseline: 1.3202x; 1.0043x over previous
"""APPNP GNN distributed Bass kernel for TRN2 (8 NeuronCores).

v4 design:
  - Row (destination-node) 1D sharding: core c owns rows [c*R, (c+1)*R).
  - Gather table [N, 64] bf16 replicated per-core in DRAM, refreshed each
    APPNP step by AllGather (1 MiB per core in, 8 MiB table out).
  - Pair-fetch gather: each 256B descriptor fetches the bf16 row PAIR
    (2i, 2i+1) with idx = col>>1 (fits int16, no LO/HI split -> fewer
    padded windows). A DVE select (parity mask) picks the right half.
  - dma_gather calls kept at <=1024 descriptors (empirical SWDGE desc-gen
    sweet spot ~2.8ns/desc).
  - Messages aggregated on TensorE: per 128-edge window a host-built
    S matrix [128 edges, 128 dests] (edge weight at the dest column) is
    the stationary operand; PSUM accumulates the segment sum.
  - MLP tail writes alpha*h0 straight into SBUF h0s and a bf16 row tile
    DMA'd to hnew for the AllGather.
"""
from contextlib import ExitStack
from dataclasses import dataclass
import math
import numpy as np
import ml_dtypes

from concourse import bass, bacc, mybir, library_config

FP = mybir.dt.float32
BF = mybir.dt.bfloat16
I16 = mybir.dt.int16
AF = mybir.ActivationFunctionType


@dataclass
class Cfg:
    N: int = 65536
    CORES: int = 8
    IN: int = 512           # padded input dim (real 500)
    HID: int = 256
    D: int = 64
    K: int = 10
    ALPHA: float = 0.1
    BPC: int = 2            # dest blocks per chunk
    WLO: int = 0            # windows per block (filled by prep); WHI kept 0
    WHI: int = 0
    DEBUG: bool = False

    @property
    def R(self):
        return self.N // self.CORES

    @property
    def NB(self):           # dest blocks per core
        return self.R // 128

    @property
    def WPB(self):
        return self.WLO + self.WHI

    @property
    def NWIN(self):         # windows per core
        return self.NB * self.WPB

    @property
    def S_SLOTS(self):      # gather slots per core
        return self.NWIN * 128

    @property
    def NCHUNK(self):
        return self.NB // self.BPC


def wrap16(a):
    m = a.reshape(-1, 16).T
    return np.tile(m, (8, 1)).copy()


def prep_inputs(cfg, x, W1, b1, W2, b2, W3, b3, edge_weight, edge_row, edge_col):
    N, R, D = cfg.N, cfg.R, cfg.D
    edge_row = np.asarray(edge_row).astype(np.int64)
    edge_col = np.asarray(edge_col).astype(np.int64)
    edge_weight = np.asarray(edge_weight).astype(np.float32)
    x = np.asarray(x)

    # global sort once: by dest block
    blk = edge_row // 128                       # global block id
    order = np.lexsort((edge_col, blk))
    er, ec, ew = edge_row[order], edge_col[order], edge_weight[order]
    gblk = blk[order]

    NBG = N // 128                              # total blocks
    cnt = np.zeros(NBG, np.int64)
    np.add.at(cnt, gblk, 1)
    cfg.WLO = max(int(np.ceil(cnt.max() / 128)), 1)
    cfg.WHI = 0

    NB, BPC, W = cfg.NB, cfg.BPC, cfg.WLO
    assert NB % BPC == 0
    CH2 = BPC * W * 128

    eye = np.eye(128, dtype=np.float32)
    W1p = np.zeros((cfg.IN, cfg.HID), np.float32)
    W1p[:W1.shape[0]] = W1

    # per-edge slot id within its core: chunk base + brel*W*128 + rank
    b_loc = gblk % NB
    chunk = b_loc // BPC
    brel = b_loc % BPC
    grp_starts = np.searchsorted(gblk, np.arange(NBG), side="left")
    rank = np.arange(len(er)) - grp_starts[gblk]
    slot = chunk * CH2 + brel * W * 128 + rank

    core = gblk // NB
    dest_rel = er % 128
    gval = (ec >> 1).astype(np.int16)
    pval = (ec & 1).astype(np.float32)

    S_SLOTS = cfg.S_SLOTS
    p_arr = (np.arange(S_SLOTS) % 128).astype(np.int64)
    w_arr = (np.arange(S_SLOTS) // 128).astype(np.int64)
    in_maps = []
    for c in range(cfg.CORES):
        m = core == c
        sl = slot[m].astype(np.int64)
        assert sl.max() < S_SLOTS
        gidx = np.zeros(S_SLOTS, np.int16)
        gidx[sl] = gval[m]
        par = np.zeros(S_SLOTS, np.float32)
        par[sl] = pval[m]
        drel = np.zeros(S_SLOTS, np.int64)
        drel[sl] = dest_rel[m]
        wt = np.zeros(S_SLOTS, np.float32)
        wt[sl] = ew[m]

        S = np.zeros((128, cfg.NWIN, 128), ml_dtypes.bfloat16)
        S[p_arr, w_arr, drel] = wt.astype(ml_dtypes.bfloat16)
        parm = np.zeros((128, cfg.NWIN), np.uint8)
        parm[p_arr, w_arr] = par.astype(np.uint8)

        xT = np.zeros((cfg.IN, R), np.float32)
        xs = np.asarray(x[c * R:(c + 1) * R])
        xT[:xs.shape[1], :] = xs.T.astype(np.float32)

        in_maps.append({
            "xT": np.ascontiguousarray(xT),
            "W1": W1p,
            "b1": np.asarray(b1).astype(np.float32).reshape(-1, 128).T.copy(),
            "W2": np.asarray(W2).astype(np.float32),
            "b2": np.asarray(b2).astype(np.float32).reshape(-1, 128).T.copy(),
            "W3": np.asarray(W3).astype(np.float32),
            "b3": np.asarray(b3).reshape(-1, 1).astype(np.float32),
            "eye": eye,
            "gidx": wrap16(gidx),
            "par": parm,
            "smat": S,
        })
    return cfg, in_maps


def build(cfg: Cfg):
    N, R, D, K = cfg.N, cfg.R, cfg.D, cfg.K
    IN, HID = cfg.IN, cfg.HID
    KIN, KH, MH = IN // 128, HID // 128, HID // 128
    NT = R // 128
    NB, BPC, W = cfg.NB, cfg.BPC, cfg.WLO
    NWC = BPC * W
    CH2 = NWC * 128
    NCH = cfg.NCHUNK
    FPB = NB * D
    MAXW = 8                 # max windows (1024 descs) per dma_gather call
    NCALL = math.ceil(NWC / MAXW)

    nc = bacc.Bacc(target_bir_lowering=False, num_devices=cfg.CORES,
                   num_swdge_queues=4)

    xT = nc.declare_dram_parameter("xT", [IN, R], FP, isOutput=False)
    W1 = nc.declare_dram_parameter("W1", [IN, HID], FP, isOutput=False)
    b1 = nc.declare_dram_parameter("b1", [128, HID // 128], FP, isOutput=False)
    W2 = nc.declare_dram_parameter("W2", [HID, HID], FP, isOutput=False)
    b2 = nc.declare_dram_parameter("b2", [128, HID // 128], FP, isOutput=False)
    W3 = nc.declare_dram_parameter("W3", [HID, D], FP, isOutput=False)
    b3 = nc.declare_dram_parameter("b3", [D, 1], FP, isOutput=False)
    eye = nc.declare_dram_parameter("eye", [128, 128], FP, isOutput=False)
    gidx = nc.declare_dram_parameter("gidx", [128, cfg.S_SLOTS // 16], I16, isOutput=False)
    par = nc.declare_dram_parameter("par", [128, cfg.NWIN], mybir.dt.uint8, isOutput=False)
    smat = nc.declare_dram_parameter("smat", [128, cfg.NWIN, 128], BF, isOutput=False)
    out = nc.declare_dram_parameter("out", [R, D], FP, isOutput=True)

    table = nc.dram_tensor("table", [N, D], BF, addr_space="Shared")
    hnew = nc.dram_tensor("hnew", [R, D], BF)

    # ---- semaphore plan (every DMA sem has <=1 DMA in flight) ----
    GATHER_SEMS = [f"g{p}{j}" for p in range(4) for j in range(NCALL)]
    SEMNAMES = ["smain", "sd0", "sd1", "sd2", "sd3", "gmain"] + GATHER_SEMS + ["v", "a", "p", "c"]
    DMA_SEMS = {"smain", "sd0", "sd1", "sd2", "sd3", "gmain", *GATHER_SEMS}
    ENG_OF = {sn: 'g' for sn in GATHER_SEMS}
    ENG_OF.update({"smain": 's', "sd0": 's', "sd1": 's', "sd2": 's', "sd3": 's', "gmain": 'g',
                   "v": 'v', "a": 'a', "p": 'p', "c": 'g'})
    sched = []      # (engine, fn, waits{semname: val}, semname)
    cnt = {sn: 0 for sn in SEMNAMES}

    def add(semname, fn, waits=None):
        sched.append((ENG_OF[semname], fn, dict(waits or {}), semname))
        cnt[semname] += 16 if semname in DMA_SEMS else 1
        return cnt[semname]

    es = ExitStack()
    with es:
        SEMH = {sn: es.enter_context(nc.semaphore("sem_" + sn)) for sn in SEMNAMES}

        gidx_sb = es.enter_context(nc.sbuf_tensor("gidx_sb", [128, cfg.S_SLOTS // 16], I16))
        par_sb = es.enter_context(nc.sbuf_tensor("par_sb", [128, cfg.NWIN], mybir.dt.uint8))
        msgb = [es.enter_context(nc.sbuf_tensor(f"msgb{i}", [128, NWC, 128], BF)) for i in range(4)]
        msel = [es.enter_context(nc.sbuf_tensor(f"msel{i}", [128, NWC, D], BF)) for i in range(4)]
        ssb = [es.enter_context(nc.sbuf_tensor(f"ssb{i}", [128, NWC * 128], BF)) for i in range(4)]
        h0s = es.enter_context(nc.sbuf_tensor("h0s", [128, FPB], FP))
        hnew_sb = es.enter_context(nc.sbuf_tensor("hnew_sb", [128, FPB], BF))
        hnu32 = es.enter_context(nc.sbuf_tensor("hnu32", [128, FPB], FP))
        h0bf2 = [es.enter_context(nc.sbuf_tensor(f"h0bf2_{i}", [128, D], BF)) for i in range(2)]
        w1_sb = es.enter_context(nc.sbuf_tensor("w1_sb", [128, KIN, HID], FP))
        w2_sb = es.enter_context(nc.sbuf_tensor("w2_sb", [128, KH, HID], FP))
        w3_sb = es.enter_context(nc.sbuf_tensor("w3_sb", [128, KH, D], FP))
        b1_sb = es.enter_context(nc.sbuf_tensor("b1_sb", [128, MH], FP))
        b2_sb = es.enter_context(nc.sbuf_tensor("b2_sb", [128, MH], FP))
        b3_sb = es.enter_context(nc.sbuf_tensor("b3_sb", [D, 1], FP))
        eye_sb = es.enter_context(nc.sbuf_tensor("eye_sb", [128, 128], FP))
        xt2 = [es.enter_context(nc.sbuf_tensor(f"xt2_{i}", [128, KIN, 128], FP)) for i in range(2)]
        h1t2 = [es.enter_context(nc.sbuf_tensor(f"h1t2_{i}", [128, KH, 128], FP)) for i in range(2)]
        h2t2 = [es.enter_context(nc.sbuf_tensor(f"h2t2_{i}", [128, KH, 128], FP)) for i in range(2)]
        h3t2 = [es.enter_context(nc.sbuf_tensor(f"h3t2_{i}", [D, 128], FP)) for i in range(2)]
        pw1 = es.enter_context(nc.psum_tensor("pw1", [128, 512], FP))
        pw2 = es.enter_context(nc.psum_tensor("pw2", [128, 512], FP))
        pw3 = es.enter_context(nc.psum_tensor("pw3", [128, 256], FP))
        ptr = es.enter_context(nc.psum_tensor("ptr", [128, 128], FP))
        ps_blk = [es.enter_context(nc.psum_tensor(f"ps_blk{i}", [128, D], FP))
                  for i in range(2 * BPC)]
        block = es.enter_context(nc.Block())

        # ---------------- uploads (chained on smain) ----------------
        prev_s = 0
        for fn in (
            lambda s: s.dma_start(out=w1_sb[:, :, :], in_=bass.AP(W1, 0, [[HID, 128], [128 * HID, KIN], [1, HID]])),
            lambda s: s.dma_start(out=w2_sb[:, :, :], in_=bass.AP(W2, 0, [[HID, 128], [128 * HID, KH], [1, HID]])),
            lambda s: s.dma_start(out=w3_sb[:, :, :], in_=bass.AP(W3, 0, [[D, 128], [128 * D, KH], [1, D]])),
            lambda s: s.dma_start(out=b1_sb[:, :], in_=b1[:, :]),
            lambda s: s.dma_start(out=b2_sb[:, :], in_=b2[:, :]),
            lambda s: s.dma_start(out=b3_sb[:, :], in_=b3[:, :]),
            lambda s: s.dma_start(out=eye_sb[:, :], in_=eye[:, :]),
            lambda s: s.dma_start(out=gidx_sb[:, :], in_=gidx[:, :]),
            lambda s: s.dma_start(out=par_sb[:, :], in_=par[:, :]),
        ):
            prev_s = add("smain", fn, {"smain": prev_s})
        UP_TOT = prev_s

        # ---------------- MLP (single serial chain) ----------------
        prev = ("smain", UP_TOT)

        def chain(semname, fn, extra=None):
            nonlocal prev
            w = {prev[0]: prev[1]}
            if extra:
                for k2, v2 in extra.items():
                    w[k2] = max(w.get(k2, 0), v2)
            val = add(semname, fn, w)
            prev = (semname, val)

        hnw_prev = 0
        for rt in range(NT):
            chain("smain", lambda s, rt=rt: s.dma_start(
                out=xt2[0][:, :, :],
                in_=bass.AP(xT, rt * 128, [[R, 128], [128 * R, KIN], [1, 128]])))
            for ht in range(MH):
                for kc in range(KIN):
                    chain("p", lambda p, ht=ht, kc=kc: p.matmul(
                        bass.AP(pw1, ht * 128, [[512, 128], [1, 128]]),
                        bass.AP(w1_sb, kc * HID + ht * 128, [[KIN * HID, 128], [1, 128]]),
                        xt2[0][:, kc, :],
                        start=(kc == 0), stop=(kc == KIN - 1)))
                chain("a", lambda a, ht=ht: a.activation(
                    h1t2[0][:, ht, :],
                    bass.AP(pw1, ht * 128, [[512, 128], [1, 128]]),
                    AF.Relu, bias=b1_sb[:, ht:ht + 1], scale=1.0))
            for ht in range(MH):
                for kc in range(KH):
                    chain("p", lambda p, ht=ht, kc=kc: p.matmul(
                        bass.AP(pw2, ht * 128, [[512, 128], [1, 128]]),
                        bass.AP(w2_sb, kc * HID + ht * 128, [[KH * HID, 128], [1, 128]]),
                        h1t2[0][:, kc, :],
                        start=(kc == 0), stop=(kc == KH - 1)))
                chain("a", lambda a, ht=ht: a.activation(
                    h2t2[0][:, ht, :],
                    bass.AP(pw2, ht * 128, [[512, 128], [1, 128]]),
                    AF.Relu, bias=b2_sb[:, ht:ht + 1], scale=1.0))
            for kc in range(KH):
                chain("p", lambda p, kc=kc: p.matmul(
                    bass.AP(pw3, 0, [[256, D], [1, 128]]),
                    bass.AP(w3_sb, kc * D, [[KH * D, 128], [1, D]]),
                    h2t2[0][:, kc, :],
                    start=(kc == 0), stop=(kc == KH - 1)))
            chain("v", lambda v: v.tensor_scalar_add(
                h3t2[0][:, :], bass.AP(pw3, 0, [[256, D], [1, 128]]), b3_sb[:, :]))
            chain("p", lambda p: p.transpose(
                bass.AP(ptr, 0, [[128, 128], [1, D]]),
                h3t2[0][:, :], eye_sb[0:D, 0:D]))
            chain("a", lambda a, rt=rt: a.activation(
                h0s[:, rt * D:(rt + 1) * D],
                bass.AP(ptr, 0, [[128, 128], [1, D]]),
                AF.Copy, scale=cfg.ALPHA))
            chain("a", lambda a: a.activation(
                h0bf2[0][:, :],
                bass.AP(ptr, 0, [[128, 128], [1, D]]),
                AF.Copy, scale=1.0))
            chain("gmain", lambda g, rt=rt: g.dma_start(
                out=bass.AP(hnew, rt * 128 * D, [[D, 128], [1, D]]),
                in_=h0bf2[0][:, :]), extra={"gmain": hnw_prev})
            hnw_prev = cnt["gmain"]

        A_MLP = cnt["a"]
        G_MLP = cnt["gmain"]

        # ---------------- APPNP steps ----------------
        mm_after_chunk = {}
        flush_v_after_block = {}
        mm_after_block = {}
        sel_after_chunk = {}
        gat_cum = {}
        hwr_val = G_MLP
        gci = 0   # global chunk counter across steps

        def emit_flush(fgb, fb, fpsum, k):
            dst = hnu32 if k == K - 1 else hnew_sb
            fv = add("v", lambda v, fb=fb, fpsum=fpsum, dst=dst: v.scalar_tensor_tensor(
                dst[:, fb * D:(fb + 1) * D], fpsum[:, :],
                1.0 - cfg.ALPHA, h0s[:, fb * D:(fb + 1) * D],
                mybir.AluOpType.mult, mybir.AluOpType.add),
                {"p": mm_after_block[fgb], "a": A_MLP})
            flush_v_after_block[fgb] = fv

        for k in range(K):
            ag_waits = {"gmain": hwr_val}
            for (p2, j2), val in gat_cum.items():
                ag_waits[f"g{p2}{j2}"] = val
            add("c", lambda g: g.collective_compute(
                "AllGather", mybir.AluOpType.bypass,
                replica_groups=[list(range(cfg.CORES))],
                ins=[hnew.ap().opt()], outs=[table.ap().opt()]), ag_waits)
            C_NOW = cnt["c"]
            pending_flush = []

            for ci in range(NCH):
                pr = gci % 4
                w_g = {"c": C_NOW}
                if mm_after_chunk.get(gci - 4) is not None:
                    w_g["p"] = mm_after_chunk[gci - 4]
                w_sel_gather = {}
                for j in range(NCALL):
                    w0 = j * MAXW
                    w1 = min(w0 + MAXW, NWC)
                    sn = f"g{pr}{j}"
                    qn = (gci * NCALL + j) % 4
                    gv = add(sn, lambda g, pr=pr, w0=w0, w1=w1, ci=ci, qn=qn: g.dma_gather(
                        out_ap=msgb[pr][:, w0:w1, :],
                        in_ap=bass.AP(table, 0, [[128, N // 2], [1, 128]]),
                        idxs_ap=gidx_sb[:, (ci * CH2 + w0 * 128) // 16:
                                        (ci * CH2 + w1 * 128) // 16],
                        num_idxs=(w1 - w0) * 128, num_idxs_reg=(w1 - w0) * 128,
                        elem_size=128, queue_num=qn,
                        single_packet=False), w_g)
                    gat_cum[(pr, j)] = gv
                    w_sel_gather[sn] = gv
                # parity select: pick odd/even row half per slot
                w_sel = dict(w_sel_gather)
                if mm_after_chunk.get(gci - 4) is not None:
                    w_sel["p"] = mm_after_chunk[gci - 4]
                selv = add("v", lambda v, pr=pr, ci=ci: v.select(
                    msel[pr][:, :, :],
                    bass.AP(par_sb, ci * NWC, [[cfg.NWIN, 128], [1, NWC], [0, D]]),
                    bass.AP(msgb[pr], D, [[NWC * 128, 128], [128, NWC], [1, D]]),
                    bass.AP(msgb[pr], 0, [[NWC * 128, 128], [128, NWC], [1, D]])),
                    w_sel)
                sel_after_chunk[gci] = selv

                w_s = {}
                if mm_after_chunk.get(gci - 4) is not None:
                    w_s["p"] = mm_after_chunk[gci - 4]
                sdv = add("sd" + str(pr), lambda s, ci=ci, pr=pr: s.dma_start(
                    out=ssb[pr][:, :],
                    in_=smat[:, ci * NWC:(ci + 1) * NWC, :]), w_s)

                for brel in range(BPC):
                    b = ci * BPC + brel
                    gb = k * NB + b
                    psum = ps_blk[((gb // BPC) % 2) * BPC + brel]
                    wins = [brel * W + j for j in range(W)]
                    for wi, w in enumerate(wins):
                        waits = {}
                        if wi == 0:
                            waits = {"v": selv, "sd" + str(pr): sdv}
                            prev_gb = gb - 2 * BPC
                            if prev_gb in flush_v_after_block:
                                waits["v"] = max(waits["v"], flush_v_after_block[prev_gb])
                        add("p", lambda p, pr=pr, w=w, psum=psum, wi=wi, nw=len(wins): p.matmul(
                            psum[:, :],
                            bass.AP(ssb[pr], w * 128, [[NWC * 128, 128], [1, 128]]),
                            bass.AP(msel[pr], w * D, [[NWC * D, 128], [1, D]]),
                            start=(wi == 0), stop=(wi == nw - 1)), waits)
                    mm_after_block[gb] = cnt["p"]
                    pending_flush.append((gb, b, psum))
                    if len(pending_flush) > 1:
                        fgb, fb, fpsum = pending_flush.pop(0)
                        emit_flush(fgb, fb, fpsum, k)
                mm_after_chunk[gci] = cnt["p"]
                gci += 1

            while pending_flush:
                fgb, fb, fpsum = pending_flush.pop(0)
                emit_flush(fgb, fb, fpsum, k)

            if k == K - 1:
                hwr_val = add("gmain", lambda g: g.dma_start(
                    out=bass.AP(out, 0, [[D, 128], [128 * D, NB], [1, D]]),
                    in_=hnu32[:, :]), {"v": cnt["v"], "gmain": hwr_val})
            else:
                hwr_val = add("gmain", lambda g: g.dma_start(
                    out=bass.AP(hnew, 0, [[D, 128], [128 * D, NB], [1, D]]),
                    in_=hnew_sb[:, :]), {"v": cnt["v"], "gmain": hwr_val})

        # ------------- emit -------------
        def walk(name):
            def run(eng):
                if name == 'g':
                    eng.load_library(library_config.mlp)
                last = {sn: 0 for sn in SEMNAMES}
                for (e, fn, waits, semname) in sched:
                    if e != name:
                        continue
                    for sk, val in waits.items():
                        if val > last[sk]:
                            eng.wait_ge(SEMH[sk], int(val))
                            last[sk] = int(val)
                    inc = 16 if semname in DMA_SEMS else 1
                    fn(eng).then_inc(SEMH[semname], inc)
                if name == 'g':
                    for sn in SEMNAMES:
                        if cnt[sn] > last[sn]:
                            eng.wait_ge(SEMH[sn], int(cnt[sn]))
            return run

        block.gpsimd(walk('g'))
        block.vector(walk('v'))
        block.sync(walk('s'))
        block.tensor(walk('p'))
        block.scalar(walk('a'))

    return nc


def reference_np(cfg, x, W1, b1, W2, b2, W3, b3, edge_weight, edge_row, edge_col):
    h = np.maximum(x @ W1 + b1, 0)
    h = np.maximum(h @ W2 + b2, 0)
    h = h @ W3 + b3
    h0 = h
    for _ in range(cfg.K):
        msg = h[edge_col] * edge_weight[:, None]
        aggv = np.zeros_like(h0)
        np.add.at(aggv, edge_row, msg)
        h = (1.0 - cfg.ALPHA) * aggv + cfg.ALPHA * h0
    return h


# ----------------------------------------------------------------------------
# Harness entry point: full inputs in, full output out.
# ----------------------------------------------------------------------------
def kernel(**inputs):
    cfg = Cfg()  # full-size defaults
    cfg, in_maps = prep_inputs(
        cfg,
        inputs["x"], inputs["W1"], inputs["b1"], inputs["W2"], inputs["b2"],
        inputs["W3"], inputs["b3"], inputs["edge_weight"],
        inputs["edge_row"], inputs["edge_col"],
    )
    nc = build(cfg)
    nc.finalize()
    from concourse.bass_utils import run_bass_kernel_spmd
    res = run_bass_kernel_spmd(nc, in_maps, core_ids=list(range(cfg.CORES)))
    outs = res.results
    return np.concatenate([o["out"] for o in outs], axis=0).astype(np.float32)


# revision 9
# speedup vs baseline: 1.4449x; 1.0945x over previous
"""APPNP GNN distributed Bass kernel for TRN2 (8 NeuronCores).

v4 design:
  - Row (destination-node) 1D sharding: core c owns rows [c*R, (c+1)*R).
  - Gather table [N, 64] bf16 replicated per-core in DRAM, refreshed each
    APPNP step by AllGather (1 MiB per core in, 8 MiB table out).
  - Pair-fetch gather: each 256B descriptor fetches the bf16 row PAIR
    (2i, 2i+1) with idx = col>>1 (fits int16, no LO/HI split -> fewer
    padded windows). A DVE select (parity mask) picks the right half.
  - dma_gather calls kept at <=1024 descriptors (empirical SWDGE desc-gen
    sweet spot ~2.8ns/desc).
  - Messages aggregated on TensorE: per 128-edge window a host-built
    S matrix [128 edges, 128 dests] (edge weight at the dest column) is
    the stationary operand; PSUM accumulates the segment sum.
  - MLP tail writes alpha*h0 straight into SBUF h0s and a bf16 row tile
    DMA'd to hnew for the AllGather.
"""
from contextlib import ExitStack
from dataclasses import dataclass
import math
import numpy as np
import ml_dtypes

from concourse import bass, bacc, mybir, library_config

FP = mybir.dt.float32
BF = mybir.dt.bfloat16
I16 = mybir.dt.int16
AF = mybir.ActivationFunctionType


@dataclass
class Cfg:
    N: int = 65536
    CORES: int = 8
    IN: int = 512           # padded input dim (real 500)
    HID: int = 256
    D: int = 64
    K: int = 10
    ALPHA: float = 0.1
    BPC: int = 2            # dest blocks per chunk
    WLO: int = 0            # windows per block (filled by prep); WHI kept 0
    WHI: int = 0
    DEBUG: bool = False

    @property
    def R(self):
        return self.N // self.CORES

    @property
    def NB(self):           # dest blocks per core
        return self.R // 128

    @property
    def WPB(self):
        return self.WLO + self.WHI

    @property
    def NWIN(self):         # windows per core
        return self.NB * self.WPB

    @property
    def S_SLOTS(self):      # gather slots per core
        return self.NWIN * 128

    @property
    def NCHUNK(self):
        return self.NB // self.BPC


def wrap16(a):
    m = a.reshape(-1, 16).T
    return np.tile(m, (8, 1)).copy()


def prep_inputs(cfg, x, W1, b1, W2, b2, W3, b3, edge_weight, edge_row, edge_col):
    N, R, D = cfg.N, cfg.R, cfg.D
    edge_row = np.asarray(edge_row).astype(np.int64)
    edge_col = np.asarray(edge_col).astype(np.int64)
    edge_weight = np.asarray(edge_weight).astype(np.float32)
    x = np.asarray(x)

    # global sort once: by dest block
    blk = edge_row // 128                       # global block id
    order = np.lexsort((edge_col, blk))
    er, ec, ew = edge_row[order], edge_col[order], edge_weight[order]
    gblk = blk[order]

    NBG = N // 128                              # total blocks
    cnt = np.zeros(NBG, np.int64)
    np.add.at(cnt, gblk, 1)
    cfg.WLO = max(int(np.ceil(cnt.max() / 128)), 1)
    cfg.WHI = 0

    NB, BPC, W = cfg.NB, cfg.BPC, cfg.WLO
    assert NB % BPC == 0
    CH2 = BPC * W * 128

    eye = np.eye(128, dtype=np.float32)
    W1p = np.zeros((cfg.IN, cfg.HID), np.float32)
    W1p[:W1.shape[0]] = W1

    # per-edge slot id within its core: chunk base + brel*W*128 + rank
    b_loc = gblk % NB
    chunk = b_loc // BPC
    brel = b_loc % BPC
    grp_starts = np.searchsorted(gblk, np.arange(NBG), side="left")
    rank = np.arange(len(er)) - grp_starts[gblk]
    slot = chunk * CH2 + brel * W * 128 + rank

    core = gblk // NB
    dest_rel = er % 128
    gval = (ec >> 1).astype(np.int16)
    pval = (ec & 1).astype(np.float32)

    S_SLOTS = cfg.S_SLOTS
    p_arr = (np.arange(S_SLOTS) % 128).astype(np.int64)
    w_arr = (np.arange(S_SLOTS) // 128).astype(np.int64)
    in_maps = []
    for c in range(cfg.CORES):
        m = core == c
        sl = slot[m].astype(np.int64)
        assert sl.max() < S_SLOTS
        gidx = np.zeros(S_SLOTS, np.int16)
        gidx[sl] = gval[m]
        par = np.zeros(S_SLOTS, np.float32)
        par[sl] = pval[m]
        drel = np.zeros(S_SLOTS, np.int64)
        drel[sl] = dest_rel[m]
        wt = np.zeros(S_SLOTS, np.float32)
        wt[sl] = ew[m]

        S = np.zeros((128, cfg.NWIN, 128), ml_dtypes.bfloat16)
        S[p_arr, w_arr, drel] = wt.astype(ml_dtypes.bfloat16)
        parm = np.zeros((128, cfg.NWIN), np.uint8)
        parm[p_arr, w_arr] = par.astype(np.uint8)

        xT = np.zeros((cfg.IN, R), np.float32)
        xs = np.asarray(x[c * R:(c + 1) * R])
        xT[:xs.shape[1], :] = xs.T.astype(np.float32)

        in_maps.append({
            "xT": np.ascontiguousarray(xT),
            "W1": W1p,
            "b1": np.asarray(b1).astype(np.float32).reshape(-1, 128).T.copy(),
            "W2": np.asarray(W2).astype(np.float32),
            "b2": np.asarray(b2).astype(np.float32).reshape(-1, 128).T.copy(),
            "W3": np.asarray(W3).astype(np.float32),
            "b3": np.asarray(b3).reshape(-1, 1).astype(np.float32),
            "eye": eye,
            "gidx": wrap16(gidx),
            "par": parm,
            "smat": S,
        })
    return cfg, in_maps


def build(cfg: Cfg):
    N, R, D, K = cfg.N, cfg.R, cfg.D, cfg.K
    IN, HID = cfg.IN, cfg.HID
    KIN, KH, MH = IN // 128, HID // 128, HID // 128
    NT = R // 128
    NB, BPC, W = cfg.NB, cfg.BPC, cfg.WLO
    NWC = BPC * W
    CH2 = NWC * 128
    NCH = cfg.NCHUNK
    FPB = NB * D
    MAXW = 8                 # max windows (1024 descs) per dma_gather call
    NCALL = math.ceil(NWC / MAXW)

    nc = bacc.Bacc(target_bir_lowering=False, num_devices=cfg.CORES,
                   num_swdge_queues=4)

    xT = nc.declare_dram_parameter("xT", [IN, R], FP, isOutput=False)
    W1 = nc.declare_dram_parameter("W1", [IN, HID], FP, isOutput=False)
    b1 = nc.declare_dram_parameter("b1", [128, HID // 128], FP, isOutput=False)
    W2 = nc.declare_dram_parameter("W2", [HID, HID], FP, isOutput=False)
    b2 = nc.declare_dram_parameter("b2", [128, HID // 128], FP, isOutput=False)
    W3 = nc.declare_dram_parameter("W3", [HID, D], FP, isOutput=False)
    b3 = nc.declare_dram_parameter("b3", [D, 1], FP, isOutput=False)
    eye = nc.declare_dram_parameter("eye", [128, 128], FP, isOutput=False)
    gidx = nc.declare_dram_parameter("gidx", [128, cfg.S_SLOTS // 16], I16, isOutput=False)
    par = nc.declare_dram_parameter("par", [128, cfg.NWIN], mybir.dt.uint8, isOutput=False)
    smat = nc.declare_dram_parameter("smat", [128, cfg.NWIN, 128], BF, isOutput=False)
    out = nc.declare_dram_parameter("out", [R, D], FP, isOutput=True)

    table = nc.dram_tensor("table", [N, D], BF, addr_space="Shared")
    hnew = nc.dram_tensor("hnew", [R, D], BF)

    # ---- semaphore plan (every DMA sem has <=1 DMA in flight) ----
    GATHER_SEMS = [f"g{p}{j}" for p in range(4) for j in range(NCALL)]
    SEMNAMES = ["smain", "sd0", "sd1", "sd2", "sd3", "gmain"] + GATHER_SEMS + ["v", "a", "p", "c"]
    DMA_SEMS = {"smain", "sd0", "sd1", "sd2", "sd3", "gmain", *GATHER_SEMS}
    ENG_OF = {sn: 'g' for sn in GATHER_SEMS}
    ENG_OF.update({"smain": 's', "sd0": 's', "sd1": 's', "sd2": 's', "sd3": 's', "gmain": 'g',
                   "v": 'v', "a": 'a', "p": 'p', "c": 'g'})
    sched = []      # (engine, fn, waits{semname: val}, semname)
    cnt = {sn: 0 for sn in SEMNAMES}

    def add(semname, fn, waits=None):
        sched.append((ENG_OF[semname], fn, dict(waits or {}), semname))
        cnt[semname] += 16 if semname in DMA_SEMS else 1
        return cnt[semname]

    es = ExitStack()
    with es:
        SEMH = {sn: es.enter_context(nc.semaphore("sem_" + sn)) for sn in SEMNAMES}

        gidx_sb = es.enter_context(nc.sbuf_tensor("gidx_sb", [128, cfg.S_SLOTS // 16], I16))
        par_sb = es.enter_context(nc.sbuf_tensor("par_sb", [128, cfg.NWIN], mybir.dt.uint8))
        msgb = [es.enter_context(nc.sbuf_tensor(f"msgb{i}", [128, NWC, 128], BF)) for i in range(4)]
        msel = [es.enter_context(nc.sbuf_tensor(f"msel{i}", [128, NWC, D], BF)) for i in range(4)]
        ssb = [es.enter_context(nc.sbuf_tensor(f"ssb{i}", [128, NWC * 128], BF)) for i in range(4)]
        h0s = es.enter_context(nc.sbuf_tensor("h0s", [128, FPB], FP))
        hnew_sb = es.enter_context(nc.sbuf_tensor("hnew_sb", [128, FPB], BF))
        hnu32 = es.enter_context(nc.sbuf_tensor("hnu32", [128, FPB], FP))
        h0bf2 = [es.enter_context(nc.sbuf_tensor(f"h0bf2_{i}", [128, D], BF)) for i in range(2)]
        w1_sb = es.enter_context(nc.sbuf_tensor("w1_sb", [128, KIN, HID], FP))
        w2_sb = es.enter_context(nc.sbuf_tensor("w2_sb", [128, KH, HID], FP))
        w3_sb = es.enter_context(nc.sbuf_tensor("w3_sb", [128, KH, D], FP))
        b1_sb = es.enter_context(nc.sbuf_tensor("b1_sb", [128, MH], FP))
        b2_sb = es.enter_context(nc.sbuf_tensor("b2_sb", [128, MH], FP))
        b3_sb = es.enter_context(nc.sbuf_tensor("b3_sb", [D, 1], FP))
        eye_sb = es.enter_context(nc.sbuf_tensor("eye_sb", [128, 128], FP))
        xt2 = [es.enter_context(nc.sbuf_tensor(f"xt2_{i}", [128, KIN, 128], FP)) for i in range(2)]
        h1t2 = [es.enter_context(nc.sbuf_tensor(f"h1t2_{i}", [128, KH, 128], FP)) for i in range(2)]
        h2t2 = [es.enter_context(nc.sbuf_tensor(f"h2t2_{i}", [128, KH, 128], FP)) for i in range(2)]
        h3t2 = [es.enter_context(nc.sbuf_tensor(f"h3t2_{i}", [D, 128], FP)) for i in range(2)]
        pb1 = [es.enter_context(nc.psum_tensor(f"pb1_{i}", [128, 512], FP)) for i in range(2)]
        pb2 = [es.enter_context(nc.psum_tensor(f"pb2_{i}", [128, 512], FP)) for i in range(2)]
        pb3 = [es.enter_context(nc.psum_tensor(f"pb3_{i}", [128, 256], FP)) for i in range(2)]
        ptr2 = [es.enter_context(nc.psum_tensor(f"ptr2_{i}", [128, 128], FP)) for i in range(2)]
        # APPNP block psums alias the spare tail columns of the MLP banks
        # (phases never overlap in time; each alias owns its whole bank here)
        ps_blk = [bass.AP(t, 448, [[512, 128], [1, D]])
                  for t in (pb1[0], pb1[1], pb2[0], pb2[1])]
        block = es.enter_context(nc.Block())

        # ---------------- uploads (chained on smain) ----------------
        prev_s = 0
        for fn in (
            lambda s: s.dma_start(out=w1_sb[:, :, :], in_=bass.AP(W1, 0, [[HID, 128], [128 * HID, KIN], [1, HID]])),
            lambda s: s.dma_start(out=w2_sb[:, :, :], in_=bass.AP(W2, 0, [[HID, 128], [128 * HID, KH], [1, HID]])),
            lambda s: s.dma_start(out=w3_sb[:, :, :], in_=bass.AP(W3, 0, [[D, 128], [128 * D, KH], [1, D]])),
            lambda s: s.dma_start(out=b1_sb[:, :], in_=b1[:, :]),
            lambda s: s.dma_start(out=b2_sb[:, :], in_=b2[:, :]),
            lambda s: s.dma_start(out=b3_sb[:, :], in_=b3[:, :]),
            lambda s: s.dma_start(out=eye_sb[:, :], in_=eye[:, :]),
            lambda s: s.dma_start(out=gidx_sb[:, :], in_=gidx[:, :]),
            lambda s: s.dma_start(out=par_sb[:, :], in_=par[:, :]),
        ):
            prev_s = add("smain", fn, {"smain": prev_s})
        UP_TOT = prev_s

        # ------------- MLP (4-stage pipeline, bank-correct PSUM) -------------
        # A: load xt + W1 mm; B: W2 mm; C: W3 mm + bias add; D: transpose +
        # h0 acts + hnew DMA. Per-parity PSUM banks; an ACT read of a bank is
        # never concurrent with a PE write to it (acts wait for both ht mms,
        # next-parity mms wait for the prior read of their bank).
        ld = {}; w1d = {}; a1d = {}; w2d = {}; a2d = {}
        w3d = {}; vad = {}; trd = {}; h0d = {}; dmad = {}
        sm_prev = UP_TOT
        for it in range(NT + 3):
            tA, tB, tC, tD = it, it - 1, it - 2, it - 3
            if tA < NT:
                w = {"smain": sm_prev}
                if tA - 2 >= 0:
                    w["p"] = w1d[tA - 2]
                sm_prev = add("smain", lambda s, t=tA: s.dma_start(
                    out=xt2[t % 2][:, :, :],
                    in_=bass.AP(xT, t * 128, [[R, 128], [128 * R, KIN], [1, 128]])), w)
                ld[tA] = sm_prev
            # ---- PE ----
            if tA < NT:
                for ht in range(MH):
                    for kc in range(KIN):
                        w = {}
                        if ht == 0 and kc == 0:
                            w["smain"] = ld[tA]
                            if tA - 2 >= 0:
                                w["a"] = a1d[tA - 2]
                        add("p", lambda p, t=tA, ht=ht, kc=kc: p.matmul(
                            bass.AP(pb1[t % 2], ht * 128, [[512, 128], [1, 128]]),
                            bass.AP(w1_sb, kc * HID + ht * 128, [[KIN * HID, 128], [1, 128]]),
                            xt2[t % 2][:, kc, :],
                            start=(kc == 0), stop=(kc == KIN - 1)), w)
                w1d[tA] = cnt["p"]
            if 0 <= tB < NT:
                for ht in range(MH):
                    for kc in range(KH):
                        w = {}
                        if kc == 0:
                            w["a"] = a1d[tB]
                            if ht == 0 and tB - 2 >= 0:
                                w["a"] = max(w["a"], a2d[tB - 2])
                        add("p", lambda p, t=tB, ht=ht, kc=kc: p.matmul(
                            bass.AP(pb2[t % 2], ht * 128, [[512, 128], [1, 128]]),
                            bass.AP(w2_sb, kc * HID + ht * 128, [[KH * HID, 128], [1, 128]]),
                            h1t2[t % 2][:, kc, :],
                            start=(kc == 0), stop=(kc == KH - 1)), w)
                w2d[tB] = cnt["p"]
            if 0 <= tC < NT:
                for kc in range(KH):
                    w = {}
                    if kc == 0:
                        w["a"] = a2d[tC]
                        if tC - 2 >= 0:
                            w["v"] = vad[tC - 2]
                    add("p", lambda p, t=tC, kc=kc: p.matmul(
                        bass.AP(pb3[t % 2], 0, [[256, D], [1, 128]]),
                        bass.AP(w3_sb, kc * D, [[KH * D, 128], [1, D]]),
                        h2t2[t % 2][:, kc, :],
                        start=(kc == 0), stop=(kc == KH - 1)), w)
                w3d[tC] = cnt["p"]
            if 0 <= tD < NT:
                w = {"v": vad[tD]}
                if tD - 2 >= 0:
                    w["a"] = h0d[tD - 2]
                add("p", lambda p, t=tD: p.transpose(
                    bass.AP(ptr2[t % 2], 0, [[128, 128], [1, D]]),
                    h3t2[t % 2][:, :], eye_sb[0:D, 0:D]), w)
                trd[tD] = cnt["p"]
            # ---- ACT ----
            if tA < NT:
                for ht in range(MH):
                    w = {"p": w1d[tA]}
                    if tA - 2 >= 0:
                        w["p"] = max(w["p"], w2d[tA - 2])
                    add("a", lambda a, t=tA, ht=ht: a.activation(
                        h1t2[t % 2][:, ht, :],
                        bass.AP(pb1[t % 2], ht * 128, [[512, 128], [1, 128]]),
                        AF.Relu, bias=b1_sb[:, ht:ht + 1], scale=1.0), w)
                a1d[tA] = cnt["a"]
            if 0 <= tB < NT:
                for ht in range(MH):
                    w = {"p": w2d[tB]}
                    if tB - 2 >= 0:
                        w["p"] = max(w["p"], w3d[tB - 2])
                    add("a", lambda a, t=tB, ht=ht: a.activation(
                        h2t2[t % 2][:, ht, :],
                        bass.AP(pb2[t % 2], ht * 128, [[512, 128], [1, 128]]),
                        AF.Relu, bias=b2_sb[:, ht:ht + 1], scale=1.0), w)
                a2d[tB] = cnt["a"]
            if 0 <= tD < NT:
                w = {"p": trd[tD]}
                add("a", lambda a, t=tD: a.activation(
                    h0s[:, t * D:(t + 1) * D],
                    bass.AP(ptr2[t % 2], 0, [[128, 128], [1, D]]),
                    AF.Copy, scale=cfg.ALPHA), w)
                w2_ = {"gmain": dmad[tD - 2]} if tD - 2 >= 0 else {}
                add("a", lambda a, t=tD: a.activation(
                    h0bf2[t % 2][:, :],
                    bass.AP(ptr2[t % 2], 0, [[128, 128], [1, D]]),
                    AF.Copy, scale=1.0), w2_)
                h0d[tD] = cnt["a"]
            # ---- DVE ----
            if 0 <= tC < NT:
                w = {"p": w3d[tC]}
                if tC - 2 >= 0:
                    w["p"] = max(w["p"], trd[tC - 2])
                add("v", lambda v, t=tC: v.tensor_scalar_add(
                    h3t2[t % 2][:, :],
                    bass.AP(pb3[t % 2], 0, [[256, D], [1, 128]]),
                    b3_sb[:, :]), w)
                vad[tC] = cnt["v"]
            # ---- hnew DMA ----
            if 0 <= tD < NT:
                w = {"a": h0d[tD]}
                if tD - 1 >= 0:
                    w["gmain"] = dmad[tD - 1]
                dmad[tD] = add("gmain", lambda g, t=tD: g.dma_start(
                    out=bass.AP(hnew, t * 128 * D, [[D, 128], [1, D]]),
                    in_=h0bf2[t % 2][:, :]), w)

        A_MLP = cnt["a"]
        G_MLP = cnt["gmain"]

        # ---------------- APPNP steps ----------------
        mm_after_chunk = {}
        flush_v_after_block = {}
        mm_after_block = {}
        sel_after_chunk = {}
        gat_cum = {}
        hwr_val = G_MLP
        gci = 0   # global chunk counter across steps

        def emit_flush(fgb, fb, fpsum, k):
            dst = hnu32 if k == K - 1 else hnew_sb
            fv = add("v", lambda v, fb=fb, fpsum=fpsum, dst=dst: v.scalar_tensor_tensor(
                dst[:, fb * D:(fb + 1) * D], fpsum,
                1.0 - cfg.ALPHA, h0s[:, fb * D:(fb + 1) * D],
                mybir.AluOpType.mult, mybir.AluOpType.add),
                {"p": mm_after_block[fgb], "a": A_MLP})
            flush_v_after_block[fgb] = fv

        for k in range(K):
            ag_waits = {"gmain": hwr_val}
            for (p2, j2), val in gat_cum.items():
                ag_waits[f"g{p2}{j2}"] = val
            add("c", lambda g: g.collective_compute(
                "AllGather", mybir.AluOpType.bypass,
                replica_groups=[list(range(cfg.CORES))],
                ins=[hnew.ap().opt()], outs=[table.ap().opt()]), ag_waits)
            C_NOW = cnt["c"]
            pending_flush = []

            for ci in range(NCH):
                pr = gci % 4
                w_g = {"c": C_NOW}
                if mm_after_chunk.get(gci - 4) is not None:
                    w_g["p"] = mm_after_chunk[gci - 4]
                w_sel_gather = {}
                for j in range(NCALL):
                    w0 = j * MAXW
                    w1 = min(w0 + MAXW, NWC)
                    sn = f"g{pr}{j}"
                    qn = (gci * NCALL + j) % 4
                    gv = add(sn, lambda g, pr=pr, w0=w0, w1=w1, ci=ci, qn=qn: g.dma_gather(
                        out_ap=msgb[pr][:, w0:w1, :],
                        in_ap=bass.AP(table, 0, [[128, N // 2], [1, 128]]),
                        idxs_ap=gidx_sb[:, (ci * CH2 + w0 * 128) // 16:
                                        (ci * CH2 + w1 * 128) // 16],
                        num_idxs=(w1 - w0) * 128, num_idxs_reg=(w1 - w0) * 128,
                        elem_size=128, queue_num=qn,
                        single_packet=False), w_g)
                    gat_cum[(pr, j)] = gv
                    w_sel_gather[sn] = gv
                # parity select: pick odd/even row half per slot
                w_sel = dict(w_sel_gather)
                if mm_after_chunk.get(gci - 4) is not None:
                    w_sel["p"] = mm_after_chunk[gci - 4]
                selv = add("v", lambda v, pr=pr, ci=ci: v.select(
                    msel[pr][:, :, :],
                    bass.AP(par_sb, ci * NWC, [[cfg.NWIN, 128], [1, NWC], [0, D]]),
                    bass.AP(msgb[pr], D, [[NWC * 128, 128], [128, NWC], [1, D]]),
                    bass.AP(msgb[pr], 0, [[NWC * 128, 128], [128, NWC], [1, D]])),
                    w_sel)
                sel_after_chunk[gci] = selv

                w_s = {}
                if mm_after_chunk.get(gci - 4) is not None:
                    w_s["p"] = mm_after_chunk[gci - 4]
                sdv = add("sd" + str(pr), lambda s, ci=ci, pr=pr: s.dma_start(
                    out=ssb[pr][:, :],
                    in_=smat[:, ci * NWC:(ci + 1) * NWC, :]), w_s)

                for brel in range(BPC):
                    b = ci * BPC + brel
                    gb = k * NB + b
                    psum = ps_blk[((gb // BPC) % 2) * BPC + brel]
                    wins = [brel * W + j for j in range(W)]
                    for wi, w in enumerate(wins):
                        waits = {}
                        if wi == 0:
                            waits = {"v": selv, "sd" + str(pr): sdv}
                            prev_gb = gb - 2 * BPC
                            if prev_gb in flush_v_after_block:
                                waits["v"] = max(waits["v"], flush_v_after_block[prev_gb])
                        add("p", lambda p, pr=pr, w=w, psum=psum, wi=wi, nw=len(wins): p.matmul(
                            psum,
                            bass.AP(ssb[pr], w * 128, [[NWC * 128, 128], [1, 128]]),
                            bass.AP(msel[pr], w * D, [[NWC * D, 128], [1, D]]),
                            start=(wi == 0), stop=(wi == nw - 1)), waits)
                    mm_after_block[gb] = cnt["p"]
                    pending_flush.append((gb, b, psum))
                    if len(pending_flush) > 1:
                        fgb, fb, fpsum = pending_flush.pop(0)
                        emit_flush(fgb, fb, fpsum, k)
                mm_after_chunk[gci] = cnt["p"]
                gci += 1

            while pending_flush:
                fgb, fb, fpsum = pending_flush.pop(0)
                emit_flush(fgb, fb, fpsum, k)

            if k == K - 1:
                hwr_val = add("gmain", lambda g: g.dma_start(
                    out=bass.AP(out, 0, [[D, 128], [128 * D, NB], [1, D]]),
                    in_=hnu32[:, :]), {"v": cnt["v"], "gmain": hwr_val})
            else:
                hwr_val = add("gmain", lambda g: g.dma_start(
                    out=bass.AP(hnew, 0, [[D, 128], [128 * D, NB], [1, D]]),
                    in_=hnew_sb[:, :]), {"v": cnt["v"], "gmain": hwr_val})

        # ------------- emit -------------
        def walk(name):
            def run(eng):
                if name == 'g':
                    eng.load_library(library_config.mlp)
                last = {sn: 0 for sn in SEMNAMES}
                for (e, fn, waits, semname) in sched:
                    if e != name:
                        continue
                    for sk, val in waits.items():
                        if val > last[sk]:
                            eng.wait_ge(SEMH[sk], int(val))
                            last[sk] = int(val)
                    inc = 16 if semname in DMA_SEMS else 1
                    fn(eng).then_inc(SEMH[semname], inc)
                if name == 'g':
                    for sn in SEMNAMES:
                        if cnt[sn] > last[sn]:
                            eng.wait_ge(SEMH[sn], int(cnt[sn]))
            return run

        block.gpsimd(walk('g'))
        block.vector(walk('v'))
        block.sync(walk('s'))
        block.tensor(walk('p'))
        block.scalar(walk('a'))

    return nc


def reference_np(cfg, x, W1, b1, W2, b2, W3, b3, edge_weight, edge_row, edge_col):
    h = np.maximum(x @ W1 + b1, 0)
    h = np.maximum(h @ W2 + b2, 0)
    h = h @ W3 + b3
    h0 = h
    for _ in range(cfg.K):
        msg = h[edge_col] * edge_weight[:, None]
        aggv = np.zeros_like(h0)
        np.add.at(aggv, edge_row, msg)
        h = (1.0 - cfg.ALPHA) * aggv + cfg.ALPHA * h0
    return h


# ----------------------------------------------------------------------------
# Harness entry point: full inputs in, full output out.
# ----------------------------------------------------------------------------
def kernel(**inputs):
    cfg = Cfg()  # full-size defaults
    cfg, in_maps = prep_inputs(
        cfg,
        inputs["x"], inputs["W1"], inputs["b1"], inputs["W2"], inputs["b2"],
        inputs["W3"], inputs["b3"], inputs["edge_weight"],
        inputs["edge_row"], inputs["edge_col"],
    )
    nc = build(cfg)
    nc.finalize()
    from concourse.bass_utils import run_bass_kernel_spmd
    res = run_bass_kernel_spmd(nc, in_maps, core_ids=list(range(cfg.CORES)))
    outs = res.results
    return np.concatenate([o["out"] for o in outs], axis=0).astype(np.float32)
